# revision 19
# baseline (speedup 1.0000x reference)
"""CKConv (SIREN continuous-kernel causal conv) Trainium2 Bass kernel.

Problem dims (hardcoded): B=32, CIN=32, COUT=32, T=2048, DK=32, K=T+1=2049.

Strategy: data-parallel over batch across 8 NeuronCores (4 samples/core).
Each core:
  1. runs the tiny SIREN kernel-net on-chip (fp32) to generate the conv
     kernel, laid out as KT3[v, i, 32*j + o] = kern[o, i, 128*(16-j) + v]
     for j in 1..16 (bf16), with column-block j=0 holding the single tap
     kern[o, i, 2048] in row v=0 (zeros elsewhere),
  2. zero-pads its x shard into xe[b,i,:] = [0]*128 ++ x ++ [0]*256 (bf16,
     staged via DRAM),
  3. loads per-(b, 8-channel-group) full-signal Hankel tiles
     H[p, i, c] = xe[b, i, p + c] (c in [0, 2048)) with one DMA of
     128x8 contiguous 4KB descriptors,
  4. computes the causal conv as Hankel x Toeplitz block matmuls: for
     window a in 15..30 the stationary operand is H[:, i, 128*(a-15):+128]
     and the moving operand is a contiguous KT3 slice; the single-tap
     rank-1 term kern[o,i,2048] is applied separately as 16 per-sample
     matmuls against natural x tiles; all matmuls of one batch sample
     accumulate directly into a single PSUM bank laid out as
     ps[u, 32*tb + o] (the j-reversal makes every window's target a
     contiguous ascending PSUM column range),
  5. adds the output bias (DVE), PE-transposes [128 u, 32 o] -> [32 o,
     128 u] per time block, and stores out[b, o, t] with one contiguous
     8KB-descriptor DMA.

The host-side wrapper only reshapes/transposes weights (pure layout) and
concatenates per-core results.
"""
import os
import numpy as np

from contextlib import ExitStack

import concourse.bass as bass
import concourse.tile as tile
from concourse import bacc, mybir
from concourse.bass_utils import run_bass_kernel_spmd
from concourse.masks import make_identity

F32 = mybir.dt.float32
BF16 = mybir.dt.bfloat16

B, CIN, COUT, T, DK = 32, 32, 32, 2048, 32
K = T + 1
L = 128
NB = T // L          # 16 time blocks
NCORES = 8
BSH = B // NCORES    # 4 batch samples per core
XE_LEN = 128 + T + 256  # 2432
HCOLS = 2048         # Hankel tile columns: window offsets for a in 15..30
IG = 8               # channels per Hankel load group
NG = CIN // IG       # 4 groups

_CACHED = {}


def _build(bsh: int = BSH):
    """Build + schedule the per-core Bass program (SPMD, no collectives)."""
    nc = bacc.Bacc(
        "TRN2", target_bir_lowering=False, debug=False, enable_asserts=False
    )

    xh = nc.dram_tensor("x", [bsh, CIN, T], F32, kind="ExternalInput")
    rph = nc.dram_tensor("rel_pos", [K], F32, kind="ExternalInput")
    w1h = nc.dram_tensor("w1", [DK], F32, kind="ExternalInput")
    b1h = nc.dram_tensor("b1", [DK], F32, kind="ExternalInput")
    om1h = nc.dram_tensor("om1", [1], F32, kind="ExternalInput")
    w2th = nc.dram_tensor("w2t", [DK, DK], F32, kind="ExternalInput")
    b2h = nc.dram_tensor("b2", [DK], F32, kind="ExternalInput")
    om2h = nc.dram_tensor("om2", [1], F32, kind="ExternalInput")
    w3ah = nc.dram_tensor("w3a", [DK + 1, CIN * COUT], F32, kind="ExternalInput")
    biash = nc.dram_tensor("bias", [COUT], F32, kind="ExternalInput")
    outh = nc.dram_tensor("out", [COUT, bsh, T], F32, kind="ExternalOutput")

    xeh_bs = [
        nc.dram_tensor(f"xe{b}", [CIN, XE_LEN], BF16) for b in range(bsh)
    ]  # internal, per-sample so Hankel loads only wait on their own slice
    taph = nc.dram_tensor("taph", [CIN * COUT], BF16)     # internal

    with tile.TileContext(nc) as tc, ExitStack() as ctx:
        singles = ctx.enter_context(tc.tile_pool(name="singles", bufs=1))
        hankp = ctx.enter_context(tc.tile_pool(name="hankp", bufs=2))
        outp = ctx.enter_context(tc.tile_pool(name="outp", bufs=2))
        kgps = ctx.enter_context(tc.tile_pool(name="kgps", bufs=2, space="PSUM"))
        mainps = ctx.enter_context(tc.tile_pool(name="mainps", bufs=4, space="PSUM"))
        trps = ctx.enter_context(tc.tile_pool(name="trps", bufs=2, space="PSUM"))

        # ---- x staging: cast to bf16 and zero-pad into per-sample xe (DRAM) ----
        # partition dim = the 128 (b, i) rows; 8KB/4KB contiguous descriptors
        x_sb = singles.tile([bsh * CIN, T], F32)
        nc.sync.dma_start(out=x_sb, in_=bass.AP(xh, 0, [[T, bsh * CIN], [1, T]]))
        xe_sb = singles.tile([bsh * CIN, T], BF16)
        nc.vector.tensor_copy(xe_sb, x_sb)
        zeros = singles.tile([CIN, 256], BF16)
        nc.vector.memset(zeros, 0.0)
        for b in range(bsh):
            nc.scalar.dma_start(
                out=bass.AP(xeh_bs[b], 0, [[XE_LEN, CIN], [1, 128]]),
                in_=zeros[:, :128],
            )
            nc.scalar.dma_start(
                out=bass.AP(xeh_bs[b], 128 + T, [[XE_LEN, CIN], [1, 256]]),
                in_=zeros,
            )
            nc.sync.dma_start(
                out=bass.AP(xeh_bs[b], 128, [[XE_LEN, CIN], [1, T]]),
                in_=xe_sb[CIN * b:CIN * (b + 1), :],
            )

        # ---- small constants / broadcasts ----
        pos_b = singles.tile([DK, K], F32)
        nc.sync.dma_start(out=pos_b, in_=bass.AP(rph, 0, [[0, DK], [1, K]]))
        w1_sb = singles.tile([DK, 1], F32)
        nc.sync.dma_start(out=w1_sb, in_=bass.AP(w1h, 0, [[1, DK], [1, 1]]))
        b1_sb = singles.tile([DK, 1], F32)
        nc.sync.dma_start(out=b1_sb, in_=bass.AP(b1h, 0, [[1, DK], [1, 1]]))
        b2_sb = singles.tile([DK, 1], F32)
        nc.sync.dma_start(out=b2_sb, in_=bass.AP(b2h, 0, [[1, DK], [1, 1]]))
        om1_sb = singles.tile([DK, 1], F32)
        nc.sync.dma_start(out=om1_sb, in_=bass.AP(om1h, 0, [[0, DK], [1, 1]]))
        om2_sb = singles.tile([DK, 1], F32)
        nc.sync.dma_start(out=om2_sb, in_=bass.AP(om2h, 0, [[0, DK], [1, 1]]))
        w2t_sb = singles.tile([DK, DK], F32)
        nc.sync.dma_start(out=w2t_sb, in_=w2th.ap())
        w3a_sb = singles.tile([DK + 1, CIN * COUT], F32)
        nc.sync.dma_start(out=w3a_sb, in_=w3ah.ap())
        bias_sb = singles.tile([L, COUT], F32)
        nc.sync.dma_start(out=bias_sb, in_=bass.AP(biash, 0, [[0, L], [1, COUT]]))
        ident = singles.tile([L, L], F32)
        make_identity(nc, ident)

        # omega-folded layer-1 params
        w1p = singles.tile([DK, 1], F32)
        nc.vector.tensor_mul(w1p, w1_sb, om1_sb)
        b1p = singles.tile([DK, 1], F32)
        nc.vector.tensor_mul(b1p, b1_sb, om1_sb)
        b2p = singles.tile([DK, 1], F32)
        nc.vector.tensor_mul(b2p, b2_sb, om2_sb)

        # ---- SIREN layer 1: h1 = sin(om1*(w1*pos + b1)) ----
        h1 = singles.tile([DK, K], F32)
        nc.scalar.activation(
            out=h1, in_=pos_b, func=mybir.ActivationFunctionType.Sin,
            bias=b1p, scale=w1p,
        )

        # ---- SIREN layer 2: h2 = sin(om2*(w2 @ h1 + b2)); augmented ones row ----
        # h2aug stored bf16 so the layer-3 matmuls run single-pass with FWL
        h2aug = singles.tile([DK + 1, K], BF16)
        nc.vector.memset(h2aug[DK:DK + 1, :], 1.0)
        for q in range(5):
            lo = 512 * q
            hi = min(K, lo + 512)
            if lo >= hi:
                break
            z2 = kgps.tile([DK, 512], F32, tag="kg")
            nc.tensor.matmul(
                out=z2[:, :hi - lo], lhsT=w2t_sb, rhs=h1[:, lo:hi],
                start=True, stop=True,
            )
            nc.scalar.activation(
                out=h2aug[0:DK, lo:hi], in_=z2[:, :hi - lo],
                func=mybir.ActivationFunctionType.Sin, bias=b2p, scale=om2_sb,
            )
        w3a_bf = singles.tile([DK + 1, CIN * COUT], BF16)
        nc.vector.tensor_copy(w3a_bf, w3a_sb)

        # ---- layer 3 -> KT (bf16), KT4[v, j, i, o] with j = 15 - c ----
        # (j outermost makes the psum->KT copies contiguous; the conv rhs
        # slice KT4[:, 0:nj, i, :] is a cheap strided AP)
        KT = singles.tile([L, 16 * CIN * COUT], BF16)
        KT4 = KT.rearrange("p (j i o) -> p j i o", j=16, i=CIN)
        for c in range(16):
            j = 15 - c
            for g4 in range(2):
                kg = kgps.tile([L, 512], F32, tag="kg")
                nc.tensor.matmul(
                    out=kg, lhsT=h2aug[:, 128 * c:128 * (c + 1)],
                    rhs=w3a_bf[:, 512 * g4:512 * (g4 + 1)],
                    start=True, stop=True,
                )
                nc.vector.tensor_copy(
                    KT4[:, j, 16 * g4:16 * (g4 + 1), :],
                    kg.rearrange("p (i o) -> p i o", i=16),
                )
        # single-tap row kern[o, i, 2048] -> tapk_sb[i, o] (via DMA reshape)
        tapflat = singles.tile([1, CIN * COUT], BF16)
        for g4 in range(2):
            tap = kgps.tile([1, 512], F32, tag="kg")
            nc.tensor.matmul(
                out=tap, lhsT=h2aug[:, T:T + 1],
                rhs=w3a_bf[:, 512 * g4:512 * (g4 + 1)],
                start=True, stop=True,
            )
            nc.vector.tensor_copy(tapflat[:, 512 * g4:512 * (g4 + 1)], tap)
        nc.sync.dma_start(out=taph.ap(), in_=tapflat)
        tapk_sb = singles.tile([CIN, COUT], BF16)
        nc.sync.dma_start(
            out=tapk_sb, in_=bass.AP(taph, 0, [[COUT, CIN], [1, COUT]])
        )

        # ---- final output staging buffer [o, b, t] ----
        final_sb = singles.tile([COUT, bsh, T], F32)

        # ---- natural x tile [i, b, t] (bf16) for the single-tap matmuls ----
        xi = singles.tile([CIN, bsh, T], BF16)
        for b in range(bsh):
            nc.scalar.dma_start(
                out=xi[:, b, :],
                in_=bass.AP(xeh_bs[b], 128, [[XE_LEN, CIN], [1, T]]),
            )

        # ---- Hankel tile loads: H[p, ii, c] = xe[b, 8g+ii, p + c] ----
        hank_tiles = {}

        def load_hank(u):
            b, g = divmod(u, NG)
            t = hankp.tile([L, IG, HCOLS], BF16, tag="hank")
            if u < 2:
                # first tiles: split halves across both queues for low latency
                half = IG // 2
                for eng, ilo in ((nc.sync, 0), (nc.scalar, half)):
                    eng.dma_start(
                        out=t[:, ilo:ilo + half, :],
                        in_=bass.AP(
                            xeh_bs[b], (IG * g + ilo) * XE_LEN,
                            [[1, L], [XE_LEN, half], [1, HCOLS]],
                        ),
                    )
            else:
                eng = nc.sync if (u % 2 == 0) else nc.scalar
                eng.dma_start(
                    out=t,
                    in_=bass.AP(
                        xeh_bs[b], IG * g * XE_LEN,
                        [[1, L], [XE_LEN, IG], [1, HCOLS]],
                    ),
                )
            hank_tiles[u] = t

        load_hank(0)
        load_hank(1)

        # ---- main loop: one PSUM bank accumulates a full batch sample ----
        ps_cur = [None]

        def finalize_tb(b, tb, ps):
            # bias add + transpose + store for one finished time block
            osb = outp.tile([L, COUT], F32, tag="osb")
            nc.vector.tensor_add(osb, ps[:, COUT * tb:COUT * (tb + 1)], bias_sb)
            pt = trps.tile([COUT, L], F32, tag="tr")
            nc.tensor.transpose(pt, osb, ident)
            nc.vector.tensor_copy(final_sb[:, b, L * tb:L * (tb + 1)], pt)

        for u in range(bsh * NG):
            b, g = divmod(u, NG)
            if u + 2 < bsh * NG:
                load_hank(u + 2)
            hank = hank_tiles.pop(u)
            if g == 0:
                ps_b = mainps.tile([L, NB * COUT], F32, tag="main")
                ps_cur[0] = ps_b
            ps = ps_cur[0]
            for a in range(15, 31):
                # kernel chunks c in [a-15, 16) as KT4 blocks j = 15-c
                # ascending; output block tb = a-15+j; psum cols contiguous
                nj = min(16, 31 - a)
                plo = COUT * (a - 15)
                for ii in range(IG):
                    i = IG * g + ii
                    nc.tensor.matmul(
                        out=ps[:, plo:plo + COUT * nj],
                        lhsT=hank[:, ii, L * (a - 15):L * (a - 15) + L],
                        rhs=KT4[:, 0:nj, i, :],
                        start=(g == 0 and a == 15 and ii == 0),
                        stop=(g == NG - 1 and a == 30 and ii == IG - 1),
                    )
                if g == NG - 1:
                    # block tb = a-15 got its last contribution: drain it
                    finalize_tb(b, a - 15, ps)
            if g == 1:
                # rank-1 single-tap term: ps[u, 32tb+o] += x[b,:,128tb+u] @ tapk
                for tb in range(NB):
                    nc.tensor.matmul(
                        out=ps[:, COUT * tb:COUT * (tb + 1)],
                        lhsT=xi[:, b, L * tb:L * (tb + 1)],
                        rhs=tapk_sb,
                        start=False,
                        stop=False,
                    )

        # ---- write out[o, b, t]: contiguous 8KB rows ----
        nc.sync.dma_start(
            out=bass.AP(outh, 0, [[bsh * T, COUT], [T, bsh], [1, T]]),
            in_=final_sb,
        )

    nc.compile()
    return nc


def _host_prep(inputs):
    """Pure-layout host prep: transposes/reshapes/concats of the weights."""
    w2t = np.ascontiguousarray(np.asarray(inputs["w2"], np.float32).T)
    w3 = np.asarray(inputs["w3"], np.float32)
    b3 = np.asarray(inputs["b3"], np.float32)
    # w3a[m, 32*i + o] = w3[o*CIN + i, m]; w3a[DK, 32*i+o] = b3[o*CIN+i]
    w3r = w3.reshape(COUT, CIN, DK)
    w3a = np.concatenate(
        [w3r.transpose(2, 1, 0).reshape(DK, CIN * COUT),
         b3.reshape(COUT, CIN).T.reshape(1, CIN * COUT)],
        axis=0,
    )
    return {
        "rel_pos": np.ascontiguousarray(np.asarray(inputs["rel_pos"], np.float32)),
        "w1": np.ascontiguousarray(np.asarray(inputs["w1"], np.float32).reshape(DK)),
        "b1": np.ascontiguousarray(np.asarray(inputs["b1"], np.float32)),
        "om1": np.asarray(inputs["omega1"], np.float32).reshape(1).copy(),
        "w2t": w2t,
        "b2": np.ascontiguousarray(np.asarray(inputs["b2"], np.float32)),
        "om2": np.asarray(inputs["omega2"], np.float32).reshape(1).copy(),
        "w3a": np.ascontiguousarray(w3a, dtype=np.float32),
        "bias": np.ascontiguousarray(np.asarray(inputs["bias"], np.float32)),
    }


def kernel(**inputs) -> np.ndarray:
    if "nc" not in _CACHED:
        _CACHED["nc"] = _build()
    nc = _CACHED["nc"]

    x = np.ascontiguousarray(np.asarray(inputs["x"], np.float32))
    shared = _host_prep(inputs)
    in_maps = []
    for c in range(NCORES):
        m = dict(shared)
        m["x"] = np.ascontiguousarray(x[c * BSH:(c + 1) * BSH])
        in_maps.append(m)

    trace = bool(int(os.environ.get("CKCONV_TRACE", "0")))
    res = run_bass_kernel_spmd(nc, in_maps, list(range(NCORES)), trace=trace)
    _CACHED["last_results"] = res
    # out dram layout is [COUT, bsh, T] -> transpose to [bsh, COUT, T]
    out = np.concatenate(
        [res.results[c]["out"].transpose(1, 0, 2) for c in range(NCORES)], axis=0
    )
    return out.astype(np.float32)


# revision 23
# speedup vs baseline: 1.3147x; 1.3147x over previous
"""CKConv (SIREN continuous-kernel causal conv) Trainium2 Bass kernel.

Problem dims (hardcoded): B=32, CIN=32, COUT=32, T=2048, DK=32, K=T+1=2049.

Strategy: data-parallel over batch across 8 NeuronCores (4 samples/core).
Each core:
  1. runs the tiny SIREN kernel-net on-chip (fp32) to generate the conv
     kernel, laid out as KT3[v, i, 32*j + o] = kern[o, i, 128*(16-j) + v]
     for j in 1..16 (bf16), with column-block j=0 holding the single tap
     kern[o, i, 2048] in row v=0 (zeros elsewhere),
  2. zero-pads its x shard into xe[b,i,:] = [0]*128 ++ x ++ [0]*256 (bf16,
     staged via DRAM),
  3. loads per-(b, 8-channel-group) full-signal Hankel tiles
     H[p, i, c] = xe[b, i, p + c] (c in [0, 2048)) with one DMA of
     128x8 contiguous 4KB descriptors,
  4. computes the causal conv as Hankel x Toeplitz block matmuls: for
     window a in 15..30 the stationary operand is H[:, i, 128*(a-15):+128]
     and the moving operand is a contiguous KT3 slice; the single-tap
     rank-1 term kern[o,i,2048] is applied separately as 16 per-sample
     matmuls against natural x tiles; all matmuls of one batch sample
     accumulate directly into a single PSUM bank laid out as
     ps[u, 32*tb + o] (the j-reversal makes every window's target a
     contiguous ascending PSUM column range),
  5. adds the output bias (DVE), PE-transposes [128 u, 32 o] -> [32 o,
     128 u] per time block, and stores out[b, o, t] with one contiguous
     8KB-descriptor DMA.

The host-side wrapper only reshapes/transposes weights (pure layout) and
concatenates per-core results.
"""
import os
import numpy as np

from contextlib import ExitStack

import concourse.bass as bass
import concourse.tile as tile
from concourse import bacc, mybir
from concourse.bass_utils import run_bass_kernel_spmd
from concourse.masks import make_identity

F32 = mybir.dt.float32
BF16 = mybir.dt.bfloat16

B, CIN, COUT, T, DK = 32, 32, 32, 2048, 32
K = T + 1
L = 128
NB = T // L          # 16 time blocks
NCORES = 8
BSH = B // NCORES    # 4 batch samples per core
XE_LEN = 128 + T     # 2176: left zero-pad + data (no right pad needed)
HCOLS = 2048         # Hankel tile columns: window offsets for a in 15..30
IG = 4               # channels per Hankel load group
NG = CIN // IG       # 8 groups

_CACHED = {}


def _build(bsh: int = BSH):
    """Build + schedule the per-core Bass program (SPMD, no collectives)."""
    nc = bacc.Bacc(
        "TRN2", target_bir_lowering=False, debug=False, enable_asserts=False
    )

    xh = nc.dram_tensor("x", [bsh, CIN, T], F32, kind="ExternalInput")
    rph = nc.dram_tensor("rel_pos", [K], F32, kind="ExternalInput")
    w1h = nc.dram_tensor("w1", [DK], F32, kind="ExternalInput")
    b1h = nc.dram_tensor("b1", [DK], F32, kind="ExternalInput")
    om1h = nc.dram_tensor("om1", [1], F32, kind="ExternalInput")
    w2th = nc.dram_tensor("w2t", [DK, DK], F32, kind="ExternalInput")
    b2h = nc.dram_tensor("b2", [DK], F32, kind="ExternalInput")
    om2h = nc.dram_tensor("om2", [1], F32, kind="ExternalInput")
    w3ah = nc.dram_tensor("w3a", [DK + 1, CIN * COUT], F32, kind="ExternalInput")
    biash = nc.dram_tensor("bias", [COUT], F32, kind="ExternalInput")
    outh = nc.dram_tensor("out", [COUT, bsh, T], F32, kind="ExternalOutput")

    xeh_bs = [
        nc.dram_tensor(f"xe{b}", [CIN, XE_LEN], BF16) for b in range(bsh)
    ]  # internal, per-sample so Hankel loads only wait on their own slice
    taph = nc.dram_tensor("taph", [CIN * COUT], BF16)     # internal

    with tile.TileContext(nc) as tc, ExitStack() as ctx:
        singles = ctx.enter_context(tc.tile_pool(name="singles", bufs=1))
        hankp = ctx.enter_context(tc.tile_pool(name="hankp", bufs=4))
        outp = ctx.enter_context(tc.tile_pool(name="outp", bufs=2))
        kgps = ctx.enter_context(tc.tile_pool(name="kgps", bufs=2, space="PSUM"))
        mainps = ctx.enter_context(tc.tile_pool(name="mainps", bufs=4, space="PSUM"))
        trps = ctx.enter_context(tc.tile_pool(name="trps", bufs=2, space="PSUM"))

        # ---- small constants (scalar queue, so staging owns the sync queue) ----
        pos_b = singles.tile([DK, K], F32)
        nc.scalar.dma_start(out=pos_b, in_=bass.AP(rph, 0, [[0, DK], [1, K]]))
        w1_sb = singles.tile([DK, 1], F32)
        nc.scalar.dma_start(out=w1_sb, in_=bass.AP(w1h, 0, [[1, DK], [1, 1]]))
        b1_sb = singles.tile([DK, 1], F32)
        nc.scalar.dma_start(out=b1_sb, in_=bass.AP(b1h, 0, [[1, DK], [1, 1]]))
        b2_sb = singles.tile([DK, 1], F32)
        nc.scalar.dma_start(out=b2_sb, in_=bass.AP(b2h, 0, [[1, DK], [1, 1]]))
        om1_sb = singles.tile([DK, 1], F32)
        nc.scalar.dma_start(out=om1_sb, in_=bass.AP(om1h, 0, [[0, DK], [1, 1]]))
        om2_sb = singles.tile([DK, 1], F32)
        nc.scalar.dma_start(out=om2_sb, in_=bass.AP(om2h, 0, [[0, DK], [1, 1]]))
        w2t_sb = singles.tile([DK, DK], F32)
        nc.scalar.dma_start(out=w2t_sb, in_=w2th.ap())
        w3a_sb = singles.tile([DK + 1, CIN * COUT], F32)
        nc.scalar.dma_start(out=w3a_sb, in_=w3ah.ap())
        bias_sb = singles.tile([L, COUT], F32)
        nc.scalar.dma_start(out=bias_sb, in_=bass.AP(biash, 0, [[0, L], [1, COUT]]))
        ident = singles.tile([L, L], F32)
        make_identity(nc, ident)

        # ---- x staging: cast to bf16, zero-pad into per-sample xe (DRAM) ----
        # partition dim = the 128 (b, i) rows; 8KB/4KB contiguous descriptors
        x_sb = singles.tile([bsh * CIN, T], F32)
        nc.sync.dma_start(out=x_sb, in_=bass.AP(xh, 0, [[T, bsh * CIN], [1, T]]))
        xe_sb = singles.tile([bsh * CIN, T], BF16)
        nc.vector.tensor_copy(xe_sb, x_sb)
        zeros = singles.tile([CIN, 128], BF16)
        nc.vector.memset(zeros, 0.0)
        for b in range(bsh):
            nc.gpsimd.dma_start(
                out=bass.AP(xeh_bs[b], 0, [[XE_LEN, CIN], [1, 128]]),
                in_=zeros,
            )
            nc.sync.dma_start(
                out=bass.AP(xeh_bs[b], 128, [[XE_LEN, CIN], [1, T]]),
                in_=xe_sb[CIN * b:CIN * (b + 1), :],
            )

        # ---- PE pre-warm: keep the array busy until real matmuls arrive so
        # the HAM clock gate reaches 8/8 before the main loop ----
        KT = singles.tile([L, 16 * CIN * COUT], BF16)
        KT4 = KT.rearrange("p (j i o) -> p j i o", j=16, i=CIN)
        nc.vector.memset(KT[:, 0:512], 0.0)
        for w in range(36):
            wps = kgps.tile([L, 512], F32, tag="kg")
            nc.tensor.matmul(
                out=wps[:, 0:128], lhsT=KT[:, 0:128], rhs=KT[:, 0:128],
                start=True, stop=True,
            )

        # omega-folded layer-1 params
        w1p = singles.tile([DK, 1], F32)
        nc.vector.tensor_mul(w1p, w1_sb, om1_sb)
        b1p = singles.tile([DK, 1], F32)
        nc.vector.tensor_mul(b1p, b1_sb, om1_sb)
        b2p = singles.tile([DK, 1], F32)
        nc.vector.tensor_mul(b2p, b2_sb, om2_sb)

        # ---- SIREN layer 1: h1 = sin(om1*(w1*pos + b1)) ----
        h1 = singles.tile([DK, K], F32)
        nc.scalar.activation(
            out=h1, in_=pos_b, func=mybir.ActivationFunctionType.Sin,
            bias=b1p, scale=w1p,
        )
        w3a_bf = singles.tile([DK + 1, CIN * COUT], BF16)
        nc.vector.tensor_copy(w3a_bf, w3a_sb)

        # ---- SIREN layer 2 + layer 3, interleaved per 512-chunk so the
        # KT blocks start filling as soon as each h2 chunk is ready ----
        # h2aug stored bf16 so the layer-3 matmuls run single-pass with FWL
        h2aug = singles.tile([DK + 1, K], BF16)
        nc.vector.memset(h2aug[DK:DK + 1, :], 1.0)
        tapflat = singles.tile([1, CIN * COUT], BF16)
        for q in range(5):
            lo = 512 * q
            hi = min(K, lo + 512)
            z2 = kgps.tile([DK, 512], F32, tag="kg")
            nc.tensor.matmul(
                out=z2[:, :hi - lo], lhsT=w2t_sb, rhs=h1[:, lo:hi],
                start=True, stop=True,
            )
            nc.scalar.activation(
                out=h2aug[0:DK, lo:hi], in_=z2[:, :hi - lo],
                func=mybir.ActivationFunctionType.Sin, bias=b2p, scale=om2_sb,
            )
            if q < 4:
                # layer 3 -> KT (bf16), KT4[v, j, i, o] with j = 15 - c:
                # j outermost makes the psum->KT copies contiguous; the conv
                # rhs slice KT4[:, 0:nj, i, :] is a cheap strided AP
                for c in range(4 * q, 4 * q + 4):
                    j = 15 - c
                    for g4 in range(2):
                        kg = kgps.tile([L, 512], F32, tag="kg")
                        nc.tensor.matmul(
                            out=kg, lhsT=h2aug[:, 128 * c:128 * (c + 1)],
                            rhs=w3a_bf[:, 512 * g4:512 * (g4 + 1)],
                            start=True, stop=True,
                        )
                        nc.vector.tensor_copy(
                            KT4[:, j, 16 * g4:16 * (g4 + 1), :],
                            kg.rearrange("p (i o) -> p i o", i=16),
                        )
            else:
                # single-tap row kern[o, i, 2048] -> tapk_sb (DMA reshape)
                for g4 in range(2):
                    tap = kgps.tile([1, 512], F32, tag="kg")
                    nc.tensor.matmul(
                        out=tap, lhsT=h2aug[:, T:T + 1],
                        rhs=w3a_bf[:, 512 * g4:512 * (g4 + 1)],
                        start=True, stop=True,
                    )
                    nc.vector.tensor_copy(
                        tapflat[:, 512 * g4:512 * (g4 + 1)], tap
                    )
        nc.gpsimd.dma_start(out=taph.ap(), in_=tapflat)
        tapk_sb = singles.tile([CIN, COUT], BF16)
        nc.gpsimd.dma_start(
            out=tapk_sb, in_=bass.AP(taph, 0, [[COUT, CIN], [1, COUT]])
        )

        # ---- final output staging buffer [o, b, t] ----
        final_sb = singles.tile([COUT, bsh, T], F32)

        # ---- Hankel tile loads: H[p, ii, c] = xe[b, IG*g+ii, p + c] ----
        hank_tiles = {}

        def load_hank(u):
            b, g = divmod(u, NG)
            t = hankp.tile([L, IG, HCOLS], BF16, tag="hank")
            if u < 2:
                # first tiles: split halves across both queues for low latency
                half = IG // 2
                for eng, ilo in ((nc.sync, 0), (nc.scalar, half)):
                    eng.dma_start(
                        out=t[:, ilo:ilo + half, :],
                        in_=bass.AP(
                            xeh_bs[b], (IG * g + ilo) * XE_LEN,
                            [[1, L], [XE_LEN, half], [1, HCOLS]],
                        ),
                    )
            else:
                eng = nc.sync if (u % 2 == 0) else nc.scalar
                eng.dma_start(
                    out=t,
                    in_=bass.AP(
                        xeh_bs[b], IG * g * XE_LEN,
                        [[1, L], [XE_LEN, IG], [1, HCOLS]],
                    ),
                )
            hank_tiles[u] = t

        load_hank(0)
        load_hank(1)
        load_hank(2)

        # ---- natural x tile [i, b, t] (bf16) for the single-tap matmuls ----
        xi = singles.tile([CIN, bsh, T], BF16)
        for b in range(bsh):
            nc.scalar.dma_start(
                out=xi[:, b, :],
                in_=bass.AP(xeh_bs[b], 128, [[XE_LEN, CIN], [1, T]]),
            )

        # ---- main loop: one PSUM bank accumulates a full batch sample ----
        ps_cur = [None]

        def finalize_tb(b, tb, ps):
            # bias add + transpose + store for one finished time block
            osb = outp.tile([L, COUT], F32, tag="osb")
            nc.vector.tensor_add(osb, ps[:, COUT * tb:COUT * (tb + 1)], bias_sb)
            pt = trps.tile([COUT, L], F32, tag="tr")
            nc.tensor.transpose(pt, osb, ident)
            nc.vector.tensor_copy(final_sb[:, b, L * tb:L * (tb + 1)], pt)

        for u in range(bsh * NG):
            b, g = divmod(u, NG)
            if u + 3 < bsh * NG:
                load_hank(u + 3)
            hank = hank_tiles.pop(u)
            if g == 0:
                ps_b = mainps.tile([L, NB * COUT], F32, tag="main")
                ps_cur[0] = ps_b
            ps = ps_cur[0]
            for a in range(15, 31):
                # kernel chunks c in [a-15, 16) as KT4 blocks j = 15-c
                # ascending; output block tb = a-15+j; psum cols contiguous
                nj = min(16, 31 - a)
                plo = COUT * (a - 15)
                for ii in range(IG):
                    i = IG * g + ii
                    nc.tensor.matmul(
                        out=ps[:, plo:plo + COUT * nj],
                        lhsT=hank[:, ii, L * (a - 15):L * (a - 15) + L],
                        rhs=KT4[:, 0:nj, i, :],
                        start=(g == 0 and a == 15 and ii == 0),
                        stop=(g == NG - 1 and a == 30 and ii == IG - 1),
                    )
                if g == NG - 1:
                    # block tb = a-15 got its last contribution: drain it
                    finalize_tb(b, a - 15, ps)
            if g == 1:
                # rank-1 single-tap term: ps[u, 32tb+o] += x[b,:,128tb+u] @ tapk
                for tb in range(NB):
                    nc.tensor.matmul(
                        out=ps[:, COUT * tb:COUT * (tb + 1)],
                        lhsT=xi[:, b, L * tb:L * (tb + 1)],
                        rhs=tapk_sb,
                        start=False,
                        stop=False,
                    )

        # ---- write out[o, b, t]: contiguous 8KB rows ----
        nc.sync.dma_start(
            out=bass.AP(outh, 0, [[bsh * T, COUT], [T, bsh], [1, T]]),
            in_=final_sb,
        )

    nc.compile()
    return nc


def _host_prep(inputs):
    """Pure-layout host prep: transposes/reshapes/concats of the weights."""
    w2t = np.ascontiguousarray(np.asarray(inputs["w2"], np.float32).T)
    w3 = np.asarray(inputs["w3"], np.float32)
    b3 = np.asarray(inputs["b3"], np.float32)
    # w3a[m, 32*i + o] = w3[o*CIN + i, m]; w3a[DK, 32*i+o] = b3[o*CIN+i]
    w3r = w3.reshape(COUT, CIN, DK)
    w3a = np.concatenate(
        [w3r.transpose(2, 1, 0).reshape(DK, CIN * COUT),
         b3.reshape(COUT, CIN).T.reshape(1, CIN * COUT)],
        axis=0,
    )
    return {
        "rel_pos": np.ascontiguousarray(np.asarray(inputs["rel_pos"], np.float32)),
        "w1": np.ascontiguousarray(np.asarray(inputs["w1"], np.float32).reshape(DK)),
        "b1": np.ascontiguousarray(np.asarray(inputs["b1"], np.float32)),
        "om1": np.asarray(inputs["omega1"], np.float32).reshape(1).copy(),
        "w2t": w2t,
        "b2": np.ascontiguousarray(np.asarray(inputs["b2"], np.float32)),
        "om2": np.asarray(inputs["omega2"], np.float32).reshape(1).copy(),
        "w3a": np.ascontiguousarray(w3a, dtype=np.float32),
        "bias": np.ascontiguousarray(np.asarray(inputs["bias"], np.float32)),
    }


def kernel(**inputs) -> np.ndarray:
    if "nc" not in _CACHED:
        _CACHED["nc"] = _build()
    nc = _CACHED["nc"]

    x = np.ascontiguousarray(np.asarray(inputs["x"], np.float32))
    shared = _host_prep(inputs)
    in_maps = []
    for c in range(NCORES):
        m = dict(shared)
        m["x"] = np.ascontiguousarray(x[c * BSH:(c + 1) * BSH])
        in_maps.append(m)

    trace = bool(int(os.environ.get("CKCONV_TRACE", "0")))
    res = run_bass_kernel_spmd(nc, in_maps, list(range(NCORES)), trace=trace)
    _CACHED["last_results"] = res
    # out dram layout is [COUT, bsh, T] -> transpose to [bsh, COUT, T]
    out = np.concatenate(
        [res.results[c]["out"].transpose(1, 0, 2) for c in range(NCORES)], axis=0
    )
    return out.astype(np.float32)


# revision 32
# speedup vs baseline: 1.3321x; 1.0133x over previous
"""CKConv (SIREN continuous-kernel causal conv) Trainium2 Bass kernel.

Problem dims (hardcoded): B=32, CIN=32, COUT=32, T=2048, DK=32, K=T+1=2049.

Strategy: data-parallel over batch across 8 NeuronCores (4 samples/core).
Each core:
  1. runs the tiny SIREN kernel-net on-chip (fp32) to generate the conv
     kernel, laid out as KT3[v, i, 32*j + o] = kern[o, i, 128*(16-j) + v]
     for j in 1..16 (bf16), with column-block j=0 holding the single tap
     kern[o, i, 2048] in row v=0 (zeros elsewhere),
  2. zero-pads its x shard into xe[b,i,:] = [0]*128 ++ x ++ [0]*256 (bf16,
     staged via DRAM),
  3. loads per-(b, 8-channel-group) full-signal Hankel tiles
     H[p, i, c] = xe[b, i, p + c] (c in [0, 2048)) with one DMA of
     128x8 contiguous 4KB descriptors,
  4. computes the causal conv as Hankel x Toeplitz block matmuls: for
     window a in 15..30 the stationary operand is H[:, i, 128*(a-15):+128]
     and the moving operand is a contiguous KT3 slice; the single-tap
     rank-1 term kern[o,i,2048] is applied separately as 16 per-sample
     matmuls against natural x tiles; all matmuls of one batch sample
     accumulate directly into a single PSUM bank laid out as
     ps[u, 32*tb + o] (the j-reversal makes every window's target a
     contiguous ascending PSUM column range),
  5. adds the output bias (DVE), PE-transposes [128 u, 32 o] -> [32 o,
     128 u] per time block, and stores out[b, o, t] with one contiguous
     8KB-descriptor DMA.

The host-side wrapper only reshapes/transposes weights (pure layout) and
concatenates per-core results.
"""
import os
import numpy as np

from contextlib import ExitStack

import concourse.bass as bass
import concourse.tile as tile
from concourse import bacc, mybir
from concourse.bass_utils import run_bass_kernel_spmd

F32 = mybir.dt.float32
BF16 = mybir.dt.bfloat16

B, CIN, COUT, T, DK = 32, 32, 32, 2048, 32
K = T + 1
L = 128
NB = T // L          # 16 time blocks
NCORES = 8
BSH = B // NCORES    # 4 batch samples per core
XE_LEN = 128 + T     # 2176: left zero-pad + data (no right pad needed)
HCOLS = 2048         # Hankel tile columns: window offsets for a in 15..30
IG = 4               # channels per Hankel load group
NG = CIN // IG       # 8 groups

_CACHED = {}


def _build(bsh: int = BSH):
    """Build + schedule the per-core Bass program (SPMD, no collectives)."""
    nc = bacc.Bacc(
        "TRN2", target_bir_lowering=False, debug=False, enable_asserts=False
    )

    xh = nc.dram_tensor("x", [bsh, CIN, T], F32, kind="ExternalInput")
    rph = nc.dram_tensor("rel_pos", [K], F32, kind="ExternalInput")
    # wsmall columns: w1, b1, b2, om1 (replicated), om2 (replicated)
    wsh = nc.dram_tensor("wsmall", [DK, 5], F32, kind="ExternalInput")
    w2th = nc.dram_tensor("w2t", [DK, DK], F32, kind="ExternalInput")
    w3ah = nc.dram_tensor("w3a", [DK + 1, CIN * COUT], F32, kind="ExternalInput")
    biash = nc.dram_tensor("bias", [COUT], F32, kind="ExternalInput")
    outh = nc.dram_tensor("out", [COUT, bsh, T], F32, kind="ExternalOutput")

    xeh_bs = [
        nc.dram_tensor(f"xe{b}", [CIN, XE_LEN], BF16) for b in range(bsh)
    ]  # internal, per-sample so Hankel loads only wait on their own slice
    taph = nc.dram_tensor("taph", [CIN * COUT], BF16)     # internal

    with tile.TileContext(nc) as tc, ExitStack() as ctx:
        singles = ctx.enter_context(tc.tile_pool(name="singles", bufs=1))
        hankp = ctx.enter_context(tc.tile_pool(name="hankp", bufs=4))
        outp = ctx.enter_context(tc.tile_pool(name="outp", bufs=2))
        kgps = ctx.enter_context(tc.tile_pool(name="kgps", bufs=2, space="PSUM"))
        mainps = ctx.enter_context(tc.tile_pool(name="mainps", bufs=4, space="PSUM"))

        # ---- PE pre-warm: keep the array busy until real matmuls arrive so
        # the HAM clock gate reaches 8/8 before the main loop ----
        KT = singles.tile([L, 16 * CIN * COUT], BF16)
        KT4 = KT.rearrange("p (j i o) -> p j i o", j=16, i=CIN)
        nc.vector.memset(KT[:, 0:128], 0.0)
        for w in range(36):
            wps = kgps.tile([L, 512], F32, tag="kg")
            nc.tensor.matmul(
                out=wps[:, 0:128], lhsT=KT[:, 0:128], rhs=KT[:, 0:128],
                start=True, stop=True,
            )

        # ---- small constants (scalar queue, so staging owns the sync queue) ----
        pos_b = singles.tile([DK, K], F32)
        nc.scalar.dma_start(out=pos_b, in_=bass.AP(rph, 0, [[0, DK], [1, K]]))
        wsmall = singles.tile([DK, 5], F32)
        nc.scalar.dma_start(out=wsmall, in_=wsh.ap())
        w2t_sb = singles.tile([DK, DK], F32)
        nc.scalar.dma_start(out=w2t_sb, in_=w2th.ap())
        w3a_sb = singles.tile([DK + 1, CIN * COUT], F32)
        nc.scalar.dma_start(out=w3a_sb, in_=w3ah.ap())
        bias_sb = singles.tile([L, COUT], F32)
        nc.gpsimd.dma_start(out=bias_sb, in_=bass.AP(biash, 0, [[0, L], [1, COUT]]))

        # ---- x staging: cast to bf16, zero-pad into per-sample xe (DRAM) ----
        # partition dim = the 128 (b, i) rows; 8KB/4KB contiguous descriptors
        x_sb = singles.tile([bsh * CIN, T], F32)
        nc.sync.dma_start(out=x_sb, in_=bass.AP(xh, 0, [[T, bsh * CIN], [1, T]]))
        xe_sb = singles.tile([bsh * CIN, T], BF16)
        nc.vector.tensor_copy(xe_sb, x_sb)
        zeros = singles.tile([CIN, 128], BF16)
        nc.vector.memset(zeros, 0.0)
        for b in range(bsh):
            nc.gpsimd.dma_start(
                out=bass.AP(xeh_bs[b], 0, [[XE_LEN, CIN], [1, 128]]),
                in_=zeros,
            )
            nc.sync.dma_start(
                out=bass.AP(xeh_bs[b], 128, [[XE_LEN, CIN], [1, T]]),
                in_=xe_sb[CIN * b:CIN * (b + 1), :],
            )

        # omega-folded layer-1 params
        w1p = singles.tile([DK, 1], F32)
        nc.vector.tensor_mul(w1p, wsmall[:, 0:1], wsmall[:, 3:4])
        b1p = singles.tile([DK, 1], F32)
        nc.vector.tensor_mul(b1p, wsmall[:, 1:2], wsmall[:, 3:4])
        b2p = singles.tile([DK, 1], F32)
        nc.vector.tensor_mul(b2p, wsmall[:, 2:3], wsmall[:, 4:5])
        om2_sb = wsmall[:, 4:5]

        # ---- SIREN layer 1: h1 = sin(om1*(w1*pos + b1)); bf16 out so
        # layer 2 runs as a single-pass bf16 matmul ----
        h1 = singles.tile([DK, K], BF16)
        nc.scalar.activation(
            out=h1, in_=pos_b, func=mybir.ActivationFunctionType.Sin,
            bias=b1p, scale=w1p,
        )
        w2t_bf = singles.tile([DK, DK], BF16)
        nc.vector.tensor_copy(w2t_bf, w2t_sb)
        w3a_bf = singles.tile([DK + 1, CIN * COUT], BF16)
        nc.vector.tensor_copy(w3a_bf, w3a_sb)

        # ---- SIREN layer 2 + layer 3, interleaved per 512-chunk so the
        # KT blocks start filling as soon as each h2 chunk is ready ----
        # h2aug stored bf16 so the layer-3 matmuls run single-pass with FWL
        h2aug = singles.tile([DK + 1, K], BF16)
        nc.vector.memset(h2aug[DK:DK + 1, :], 1.0)
        tapflat = singles.tile([1, CIN * COUT], BF16)
        for q in range(5):
            lo = 512 * q
            hi = min(K, lo + 512)
            z2 = kgps.tile([DK, 512], F32, tag="kg")
            nc.tensor.matmul(
                out=z2[:, :hi - lo], lhsT=w2t_bf, rhs=h1[:, lo:hi],
                start=True, stop=True,
            )
            nc.scalar.activation(
                out=h2aug[0:DK, lo:hi], in_=z2[:, :hi - lo],
                func=mybir.ActivationFunctionType.Sin, bias=b2p, scale=om2_sb,
            )
            if q < 4:
                # layer 3 -> KT (bf16), KT4[v, j, i, o] with j = 15 - c:
                # j outermost makes the psum->KT copies contiguous; the conv
                # rhs slice KT4[:, 0:nj, i, :] is a cheap strided AP
                for c in range(4 * q, 4 * q + 4):
                    j = 15 - c
                    for g4 in range(2):
                        kg = kgps.tile([L, 512], F32, tag="kg")
                        nc.tensor.matmul(
                            out=kg, lhsT=h2aug[:, 128 * c:128 * (c + 1)],
                            rhs=w3a_bf[:, 512 * g4:512 * (g4 + 1)],
                            start=True, stop=True,
                        )
                        nc.vector.tensor_copy(
                            KT4[:, j, 16 * g4:16 * (g4 + 1), :],
                            kg.rearrange("p (i o) -> p i o", i=16),
                        )
            else:
                # single-tap row kern[o, i, 2048] -> tapk_sb (DMA reshape)
                for g4 in range(2):
                    tap = kgps.tile([1, 512], F32, tag="kg")
                    nc.tensor.matmul(
                        out=tap, lhsT=h2aug[:, T:T + 1],
                        rhs=w3a_bf[:, 512 * g4:512 * (g4 + 1)],
                        start=True, stop=True,
                    )
                    nc.vector.tensor_copy(
                        tapflat[:, 512 * g4:512 * (g4 + 1)], tap
                    )
        nc.gpsimd.dma_start(out=taph.ap(), in_=tapflat)
        tapk_sb = singles.tile([CIN, COUT], BF16)
        nc.gpsimd.dma_start(
            out=tapk_sb, in_=bass.AP(taph, 0, [[COUT, CIN], [1, COUT]])
        )

        # ---- final output staging buffer [o, b, t] ----
        final_sb = singles.tile([COUT, bsh, T], F32)

        # ---- Hankel tile loads: H[p, ii, c] = xe[b, IG*g+ii, p + c] ----
        hank_tiles = {}

        def load_hank(u):
            b, g = divmod(u, NG)
            t = hankp.tile([L, IG, HCOLS], BF16, tag="hank")
            if u < 2:
                # first tiles: split halves across both queues for low latency
                half = IG // 2
                for eng, ilo in ((nc.sync, 0), (nc.scalar, half)):
                    eng.dma_start(
                        out=t[:, ilo:ilo + half, :],
                        in_=bass.AP(
                            xeh_bs[b], (IG * g + ilo) * XE_LEN,
                            [[1, L], [XE_LEN, half], [1, HCOLS]],
                        ),
                    )
            else:
                eng = nc.sync if (u % 2 == 0) else nc.scalar
                eng.dma_start(
                    out=t,
                    in_=bass.AP(
                        xeh_bs[b], IG * g * XE_LEN,
                        [[1, L], [XE_LEN, IG], [1, HCOLS]],
                    ),
                )
            hank_tiles[u] = t

        load_hank(0)
        load_hank(1)
        load_hank(2)

        # ---- natural x tile [i, b, t] (bf16) for the single-tap matmuls ----
        xi = singles.tile([CIN, bsh, T], BF16)
        for b in range(bsh):
            nc.scalar.dma_start(
                out=xi[:, b, :],
                in_=bass.AP(xeh_bs[b], 128, [[XE_LEN, CIN], [1, T]]),
            )

        # ---- main loop: one PSUM bank accumulates a full batch sample ----
        ps_cur = [None]

        def finalize_tb(b, tb, ps):
            # bias add + 32x32 DVE stream-transposes + store for one block
            osb = outp.tile([L, COUT], F32, tag="osb")
            nc.vector.tensor_add(osb, ps[:, COUT * tb:COUT * (tb + 1)], bias_sb)
            for s in range(4):
                nc.vector.transpose(
                    final_sb[:, b, L * tb + 32 * s:L * tb + 32 * (s + 1)],
                    osb[32 * s:32 * (s + 1), :],
                )

        for u in range(bsh * NG):
            b, g = divmod(u, NG)
            if u + 3 < bsh * NG:
                load_hank(u + 3)
            hank = hank_tiles.pop(u)
            if g == 0:
                ps_b = mainps.tile([L, NB * COUT], F32, tag="main")
                ps_cur[0] = ps_b
            ps = ps_cur[0]
            for a in range(15, 31):
                # kernel chunks c in [a-15, 16) as KT4 blocks j = 15-c
                # ascending; output block tb = a-15+j; psum cols contiguous
                nj = min(16, 31 - a)
                plo = COUT * (a - 15)
                for ii in range(IG):
                    i = IG * g + ii
                    nc.tensor.matmul(
                        out=ps[:, plo:plo + COUT * nj],
                        lhsT=hank[:, ii, L * (a - 15):L * (a - 15) + L],
                        rhs=KT4[:, 0:nj, i, :],
                        start=(g == 0 and a == 15 and ii == 0),
                        stop=(g == NG - 1 and a == 30 and ii == IG - 1),
                    )
                if g == NG - 1:
                    # block tb = a-15 got its last contribution: drain it
                    finalize_tb(b, a - 15, ps)
                    if a == 30:
                        # stream this sample's finished output to DRAM
                        eng = nc.sync if (b % 2 == 0) else nc.scalar
                        eng.dma_start(
                            out=bass.AP(
                                outh, b * T, [[bsh * T, COUT], [1, T]]
                            ),
                            in_=final_sb[:, b, :],
                        )
            if g == 1:
                # rank-1 single-tap term: ps[u, 32tb+o] += x[b,:,128tb+u] @ tapk
                for tb in range(NB):
                    nc.tensor.matmul(
                        out=ps[:, COUT * tb:COUT * (tb + 1)],
                        lhsT=xi[:, b, L * tb:L * (tb + 1)],
                        rhs=tapk_sb,
                        start=False,
                        stop=False,
                    )

    nc.compile()
    return nc


def _host_prep(inputs):
    """Pure-layout host prep: transposes/reshapes/concats of the weights."""
    w2t = np.ascontiguousarray(np.asarray(inputs["w2"], np.float32).T)
    w3 = np.asarray(inputs["w3"], np.float32)
    b3 = np.asarray(inputs["b3"], np.float32)
    # w3a[m, 32*i + o] = w3[o*CIN + i, m]; w3a[DK, 32*i+o] = b3[o*CIN+i]
    w3r = w3.reshape(COUT, CIN, DK)
    w3a = np.concatenate(
        [w3r.transpose(2, 1, 0).reshape(DK, CIN * COUT),
         b3.reshape(COUT, CIN).T.reshape(1, CIN * COUT)],
        axis=0,
    )
    om1 = float(np.asarray(inputs["omega1"], np.float32).reshape(()))
    om2 = float(np.asarray(inputs["omega2"], np.float32).reshape(()))
    wsmall = np.stack(
        [
            np.asarray(inputs["w1"], np.float32).reshape(DK),
            np.asarray(inputs["b1"], np.float32).reshape(DK),
            np.asarray(inputs["b2"], np.float32).reshape(DK),
            np.full(DK, om1, np.float32),
            np.full(DK, om2, np.float32),
        ],
        axis=1,
    )
    return {
        "rel_pos": np.ascontiguousarray(np.asarray(inputs["rel_pos"], np.float32)),
        "wsmall": np.ascontiguousarray(wsmall),
        "w2t": w2t,
        "w3a": np.ascontiguousarray(w3a, dtype=np.float32),
        "bias": np.ascontiguousarray(np.asarray(inputs["bias"], np.float32)),
    }


def kernel(**inputs) -> np.ndarray:
    if "nc" not in _CACHED:
        _CACHED["nc"] = _build()
    nc = _CACHED["nc"]

    x = np.ascontiguousarray(np.asarray(inputs["x"], np.float32))
    shared = _host_prep(inputs)
    in_maps = []
    for c in range(NCORES):
        m = dict(shared)
        m["x"] = np.ascontiguousarray(x[c * BSH:(c + 1) * BSH])
        in_maps.append(m)

    trace = bool(int(os.environ.get("CKCONV_TRACE", "0")))
    res = run_bass_kernel_spmd(nc, in_maps, list(range(NCORES)), trace=trace)
    _CACHED["last_results"] = res
    # out dram layout is [COUT, bsh, T] -> transpose to [bsh, COUT, T]
    out = np.concatenate(
        [res.results[c]["out"].transpose(1, 0, 2) for c in range(NCORES)], axis=0
    )
    return out.astype(np.float32)
